# revision 29
# baseline (speedup 1.0000x reference)
"""FAST multi-head attention (p=2 Taylor linear attention) for Trainium2, v2.

Self-contained: accepts FULL inputs q,k,v [2,16,4096,32] fp32, returns the
full output [2,16,4096,32]. Shards the 32 (b,h) pairs across 8 NeuronCores
(4 per core), one Bass/Tile kernel run SPMD via PJRT.

Per (b,h) (A0=1, A1=1, A2=0.5), with v' = [v | 1]:
  out[n, :] = num / den where [num|den](n, e) = sum_f phi_f(q_n) KV[f, e],
  KV[f, e] = sum_m phi_f(k_m) v'[m, e], over an exact feature basis:
  ones(1) + linear q (32) + diag q^2 (32) + 496 off-diag pair products
  (gaps 1..15 cyclic + half gap 16). Off-diag phi(q) evaluated via the
  square trick phi = (E^T q)^2 with diag contamination removed through
  corrected diag weights wd = KV2dd - Hm @ KVc.

PE array packing (tile_position): kv-side and ans matmuls are (128,64)
col-tiled pairs (2 concurrent M=34 matmuls, one PSUM bank); u = E^T q runs
as (32,64) 8-way tiles (4 a-chunks x 2 feature halves) into a 4-bank PSUM
mega tile. Squares u^2 drain via wide-FD ScalarE ACTIVATE (plus a DVE
copy+mul path) into bf16 phit. Weight prep (p2) uses regular (128,64)
matmuls against a stacked identity, which also merges the two col-tile
partial sums. Interleaved accumulation chains in one bank use start=True
only on the very first matmul of the bank (has_written bits give
overwrite-on-first-touch for the other chain). Host pretransposes q (qt4,
qd incl. A2 q^2 rows) and lays out k with a 16-col wraparound duplicate
for stride-1 DVE products.
"""
import dataclasses
import numpy as np

import concourse.bass as bass
import concourse.tile as tile
from concourse import mybir, bacc
from concourse.bass_utils import run_bass_kernel_spmd

F32 = mybir.dt.float32
BF16 = mybir.dt.bfloat16
SQ = mybir.ActivationFunctionType.Square

A0, A1, A2 = 1.0, 1.0, 0.5
B, H, N, D = 2, 16, 4096, 32
NCORES = 8
BH_PER_CORE = (B * H) // NCORES   # 4
NT = N // 128                     # 32 n-tiles
E1 = 34                           # 32 num + 1 den + 1 pad
NOFF = 496                        # 480 (gaps 1..15) + 16 (half gap 16)

# kt feature-column layout (562 cols):
#  [0:32 k | 32:512 off gaps1-15 | 512:528 half16 | 528:529 ones |
#   529:561 diag | 561:562 pad]
KA_W, KB_W = 288, 274  # ka = kt cols 0:288, kb = kt cols 288:562

# square waves (s,h) routed to the DVE copy+mul path instead of ScalarE
SQ_ON_DVE = set()


def _ap_free(x: bass.AP, free_ap, extra_offset=0):
    return dataclasses.replace(
        x, offset=x.offset + extra_offset,
        ap=[x.ap[0]] + [list(p) for p in free_ap])


def _pairs():
    ps = []
    for g in range(1, 16):
        for d in range(D):
            ps.append((d, (d + g) % D))
    for d in range(16):
        ps.append((d, d + 16))
    return ps


def _host_consts():
    import ml_dtypes
    P = _pairs()
    beta = np.sqrt(A2)
    E = np.zeros((D, NOFF), np.float32)
    Hm = np.zeros((D, NOFF), np.float32)
    for f, (d1, d2) in enumerate(P):
        E[d1, f] += beta
        E[d2, f] += beta
        Hm[d1, f] += 1.0
        Hm[d2, f] += 1.0
    erep = np.zeros((128, 512), np.float32)
    for a in range(4):
        erep[32 * a:32 * a + 32, 0:NOFF] = E
    # hmtT[f', s, 32+d] = Hm[d, 128*s + f']  (d lands at rows 32:64 of out)
    hmtT = np.zeros((128, 4, 64), np.float32)
    for s in range(4):
        w = min(128, NOFF - 128 * s)
        hmtT[0:w, s, 32:64] = -Hm[:, 128 * s:128 * s + w].T
    # isum [128, 64]: rows 0:34 I34, rows 64:98 I34 (merges col-tile halves)
    isum = np.zeros((128, 64), np.float32)
    for m in range(E1):
        isum[m, m] = 1.0
        isum[64 + m, m] = 1.0
    return (erep.astype(ml_dtypes.bfloat16), hmtT.astype(ml_dtypes.bfloat16),
            isum.astype(ml_dtypes.bfloat16))


def build_nc():
    nc = bacc.Bacc(None, target_bir_lowering=False)

    qt4_in = nc.declare_dram_parameter("qt4", [BH_PER_CORE, 128, 1024], BF16,
                                       isOutput=False)
    qd_in = nc.declare_dram_parameter("qd", [BH_PER_CORE, 128, 2, 1024], BF16,
                                      isOutput=False)
    kpr_in = nc.declare_dram_parameter("kpr", [BH_PER_CORE, 128, NT, 80], BF16,
                                       isOutput=False)
    erep_in = nc.declare_dram_parameter("erep", [128, 512], BF16,
                                        isOutput=False)
    hmt_in = nc.declare_dram_parameter("hmt", [128, 4, 64], BF16,
                                       isOutput=False)
    isum_in = nc.declare_dram_parameter("isum", [128, 64], BF16,
                                        isOutput=False)
    ones_in = nc.declare_dram_parameter("ones1", [16, 4, 1024], BF16,
                                        isOutput=False)
    kgap_in = nc.declare_dram_parameter("kgap", [BH_PER_CORE, 128, NT, 160],
                                        BF16, isOutput=False)
    out = nc.declare_dram_parameter("out", [BH_PER_CORE, 2, 98, 1024], BF16,
                                    isOutput=True)

    with tile.TileContext(nc) as tc:
        with (
            tc.tile_pool(name="sb_const", bufs=1) as sb_const,
            tc.tile_pool(name="sb_q", bufs=2) as sb_q,
            tc.tile_pool(name="sb_kp", bufs=2) as sb_kp,
            tc.tile_pool(name="sb_w", bufs=2) as sb_w,
            tc.tile_pool(name="sb_ucp", bufs=2) as sb_ucp,
            tc.tile_pool(name="sb_ev", bufs=2) as sb_ev,
            tc.tile_pool(name="ps_kv", bufs=1, space="PSUM") as ps_kv,
            tc.tile_pool(name="ps_mega", bufs=1, space="PSUM") as ps_mega,
            tc.tile_pool(name="ps_ans", bufs=1, space="PSUM") as ps_ans,
        ):
            # ---- constants ----
            erep = sb_const.tile([128, 512], BF16)
            nc.sync.dma_start(out=erep[:], in_=erep_in[:])
            hmt = sb_const.tile([128, 4, 64], BF16)
            nc.sync.dma_start(out=hmt[:], in_=hmt_in[:])
            isum = sb_const.tile([128, 64], BF16)
            nc.sync.dma_start(out=isum[:], in_=isum_in[:])

            kts = [sb_const.tile([128, 8, 562], BF16, name=f"kts{i}")
                   for i in range(4)]
            vxs = [sb_const.tile([128, 8, E1], BF16, name=f"vxs{i}")
                   for i in range(4)]
            for i in range(4):
                nc.gpsimd.memset(kts[i][:, :, 528:529], 1.0)
                nc.gpsimd.memset(kts[i][:, :, 561:562], 0.0)
                nc.gpsimd.memset(vxs[i][:, :, 32:34], 1.0)
            phits = [sb_const.tile([128, 4, 4, 1024], BF16, name=f"phit{i}")
                     for i in range(2)]
            for i in range(2):
                nc.sync.dma_start(out=phits[i][112:128, :, 3, :],
                                  in_=ones_in[:])
            megat = ps_mega.tile([128, 2048], F32, name="megat")
            wqd_ev = sb_const.tile([128, E1], BF16, name="wqd_ev")
            wqd_od = sb_const.tile([128, E1], BF16, name="wqd_od")
            nc.gpsimd.memset(wqd_ev[:], 0.0)
            nc.gpsimd.memset(wqd_od[:], 0.0)

            def ktvx(b, g):
                return kts[g], vxs[g]

            def emit_kgap(b, g_):
                kt, _ = ktvx(b, g_)
                nc.sync.dma_start(out=kt[:, :, 352:512],
                                  in_=kgap_in[b][:, 8 * g_:8 * g_ + 8, :])

            def emit_dma_in(b, first=False):
                st = dict(b=b, phit=phits[b % 2])
                st["kpr"] = sb_kp.tile([128, NT, 80], BF16, tag="kpr",
                                       name=f"kpr_{b}")
                for g_ in range(4):
                    ts8 = slice(8 * g_, 8 * g_ + 8)
                    nc.sync.dma_start(out=st["kpr"][:, ts8, :],
                                      in_=kpr_in[b][:, ts8, :])
                    if first:
                        emit_kgap(b, g_)
                st["qt4"] = sb_q.tile([128, 1024], BF16, tag="qt4",
                                      name=f"qt4_{b}")
                nc.gpsimd.dma_start(out=st["qt4"][:], in_=qt4_in[b])
                st["qd"] = sb_q.tile([128, 2, 1024], BF16, tag="qd",
                                     name=f"qd_{b}")
                nc.gpsimd.dma_start(out=st["qd"][:], in_=qd_in[b])
                st["ka"] = ps_kv.tile([128, 512], F32, tag="ka",
                                      name=f"ka{b}")
                st["kb"] = ps_kv.tile([128, 512], F32, tag="kb",
                                      name=f"kb{b}")
                return st

            def emit_p1_group(cur, g):
                b = cur["b"]
                kt, vx = ktvx(b, g)
                ts8 = slice(8 * g, 8 * g + 8)
                kp = cur["kpr"]
                nc.vector.tensor_copy(kt[:, :, 0:32], kp[:, ts8, 0:32])
                nc.vector.tensor_copy(vx[:, :, 0:32], kp[:, ts8, 48:80])
                # off-diag products, gaps 1..15 -> kt cols 32:512
                nc.vector.tensor_mul(
                    _ap_free(kt[:, 0, 32:352], [[562, 8], [32, 10], [1, 32]]),
                    _ap_free(kp[:, 8 * g, 0:32], [[80, 8], [0, 10], [1, 32]]),
                    _ap_free(kp[:, 8 * g, 1:33], [[80, 8], [1, 10], [1, 32]]))
                # half gap 16 -> cols 512:528 ; diag -> cols 529:561
                nc.vector.tensor_mul(kt[:, :, 512:528], kp[:, ts8, 0:16],
                                     kp[:, ts8, 16:32])
                nc.vector.tensor_mul(kt[:, :, 529:561], kp[:, ts8, 0:32],
                                     kp[:, ts8, 0:32])
                for tt in range(8):
                    t = 8 * g + tt
                    par = (t % 2) * 64
                    st_, sp = (t < 2), (t >= NT - 2)
                    nc.tensor.matmul(cur["ka"][par:par + E1, 0:KA_W],
                                     vx[:, tt, :], kt[:, tt, 0:KA_W],
                                     start=st_, stop=sp,
                                     skip_group_check=True)
                    nc.tensor.matmul(cur["kb"][par:par + E1, 0:KB_W],
                                     vx[:, tt, :], kt[:, tt, KA_W:562],
                                     start=st_, stop=sp,
                                     skip_group_check=True)

            def emit_u_group(cur, s, h):
                b = cur["b"]
                qt4 = cur["qt4"]
                hs = slice(512 * h, 512 * h + 512)
                pp = 128 if s < 3 else 112
                phit = cur["phit"]
                for a in range(4):
                    for fh in range(2):
                        fw = 64 if (s < 3 or fh == 0) else 48
                        fc = 128 * s + 64 * fh
                        nc.tensor.matmul(
                            megat[64 * fh:64 * fh + fw,
                                  512 * a:512 * a + 512],
                            erep[32 * a:32 * a + 32, fc:fc + fw],
                            qt4[32 * a:32 * a + 32, hs],
                            start=True, stop=True,
                            tile_position=(32 * a, 64 * fh),
                            skip_group_check=True)
                dst = _ap_free(
                    phit[0:pp, 0, s, 512 * h:512 * h + 512],
                    [[4096, 4], [1, 512]])
                src = _ap_free(megat[0:pp, 0:512], [[512, 4], [1, 512]])
                nc.scalar.activation(out=dst, in_=src, func=SQ,
                                     scale=1.0)

            def emit_ans_chunk(st, pair, h):
                b = st["b"]
                if h == 0:
                    st["ansm%d" % pair] = ps_ans.tile(
                        [128, 1024], F32, tag="ansm", name=f"ansm{b}_{pair}")
                ansm = st["ansm%d" % pair]
                hs = slice(512 * h, 512 * h + 512)
                phit = st["phit"]
                wc = st["wc"]
                for s in range(4):
                    for ci in range(2):
                        a = 2 * pair + ci
                        nc.tensor.matmul(
                            ansm[64 * ci:64 * ci + E1, hs],
                            wc[:, 34 * s:34 * s + 34], phit[:, a, s, hs],
                            start=(s == 0), stop=False,
                            skip_group_check=True)
                for ci in range(2):
                    nc.tensor.matmul(
                        ansm[64 * ci:64 * ci + E1, hs],
                        wqd_ev if ci == 0 else wqd_od,
                        st["qd"][:, pair, hs],
                        start=False, stop=(ci == 1), skip_group_check=True)

            def emit_evict(st, pair):
                b = st["b"]
                ev = sb_ev.tile([128, 1024], BF16, tag="ev",
                                name=f"ev{b}_{pair}")
                nc.vector.tensor_copy(ev[0:98, :], st["ansm%d" % pair][0:98, :])
                nc.gpsimd.dma_start(out=out[b][pair], in_=ev[0:98, :])

            def emit_p2(cur):
                b = cur["b"]
                cov = sb_w.tile([128, KA_W], BF16, tag="cov", name=f"cov{b}")
                nc.vector.tensor_copy(cov[:], cur["ka"][:, 0:KA_W])
                cov2 = sb_w.tile([128, KB_W], BF16, tag="cov2",
                                 name=f"cov2{b}")
                nc.vector.tensor_copy(cov2[:], cur["kb"][:, 0:KB_W])

                wcp = cur["ka"]
                # wc chunks (merged col-tile halves via isum):
                srcs = [(cov, 32), (cov, 96), (cov, 160), (cov, 224),
                        (cov2, 0), (cov2, 64), (cov2, 128), (cov2, 192)]
                for i, (sc, c0) in enumerate(srcs):
                    s_, fh = i // 2, i % 2
                    w = 49 if i == 7 else 64
                    nc.tensor.matmul(wcp[64 * fh:64 * fh + w,
                                         288 + 34 * s_:288 + 34 * s_ + 34],
                                     sc[:, c0:c0 + w], isum[:, 0:34],
                                     start=True, stop=False,
                                     skip_group_check=True)
                # wq rows 0:32 at cols 170:204 ; diagT rows 32:64 at 204:238
                nc.tensor.matmul(wcp[0:64, 424:458], cov[:, 0:64],
                                 isum[:, 0:34], start=True, stop=False,
                                 skip_group_check=True)
                nc.tensor.matmul(wcp[0:64, 458:492], cov2[:, 209:273],
                                 isum[:, 0:34], start=True, stop=False,
                                 skip_group_check=True)
                wc = sb_w.tile([128, 136], BF16, tag="wc", name=f"wc{b}")
                nc.vector.tensor_copy(wc[:], wcp[:, 288:424])
                # wd = diagT - Hm @ KVc: hmt is negated, accumulate onto diag
                for s_ in range(4):
                    nc.tensor.matmul(wcp[0:64, 458:492], hmt[:, s_, :],
                                     wc[:, 34 * s_:34 * s_ + 34],
                                     start=False, stop=(s_ == 3),
                                     skip_group_check=True)
                wdt = sb_w.tile([64, E1], BF16, tag="wdt", name=f"wdt{b}")
                nc.vector.tensor_copy(wdt[32:64, :], wcp[32:64, 458:492])
                wqs = sb_w.tile([32, E1], BF16, tag="wqs", name=f"wqs{b}")
                nc.vector.tensor_copy(wqs[:], wcp[0:32, 424:458])
                cur["wdt"] = wdt
                cur["wqs"] = wqs
                cur["wcp"] = wcp
                cur["wc"] = wc

            def emit_p2b(cur):
                # assemble wqd masks (after prev's last ans chunk read them)
                nc.vector.tensor_copy(wqd_ev[0:32, :], cur["wcp"][0:32,
                                                                  424:458])
                nc.gpsimd.dma_start(out=wqd_od[32:64, :], in_=cur["wqs"][:])
                nc.gpsimd.dma_start(out=wqd_ev[64:96, :],
                                    in_=cur["wdt"][32:64, :])
                nc.gpsimd.dma_start(out=wqd_od[96:128, :],
                                    in_=cur["wdt"][32:64, :])

            # ---------------- main pipeline over b ----------------
            prev = None
            nxt = emit_dma_in(0, first=True)
            for b in range(BH_PER_CORE + 1):
                cur = nxt if b < BH_PER_CORE else None
                nxt = None
                for gi in range(8):
                    g, h = gi // 2, gi % 2
                    if cur is not None and h == 0:
                        emit_p1_group(cur, g)
                    if prev is not None and h == 1:
                        c = gi // 2
                        emit_ans_chunk(prev, c // 2, c % 2)
                        if c % 2 == 1:
                            emit_evict(prev, c // 2)
                    if cur is not None:
                        emit_u_group(cur, g, h)
                    if cur is not None and gi == 6:
                        emit_p2(cur)
                    if cur is not None and b + 1 < BH_PER_CORE:
                        if gi == 3:
                            nxt = emit_dma_in(b + 1)
                            emit_kgap(b + 1, 0)
                            emit_kgap(b + 1, 1)
                        elif gi == 5:
                            emit_kgap(b + 1, 2)
                        elif gi == 7:
                            emit_kgap(b + 1, 3)
                if cur is not None:
                    emit_p2b(cur)
                prev = cur

    nc.compile()
    return nc


_NC_CACHE = None


def _get_nc():
    global _NC_CACHE
    if _NC_CACHE is None:
        _NC_CACHE = build_nc()
    return _NC_CACHE


def _in_maps(q, k, v):
    import ml_dtypes
    BH = B * H
    qf = q.reshape(BH, N, D)
    kf = k.reshape(BH, N, D)
    vf = v.reshape(BH, N, D)
    qt = qf.reshape(BH, 4, 1024, D).transpose(0, 1, 3, 2)  # [bh, a, d, 1024]
    qt4 = np.ascontiguousarray(qt.reshape(BH, 128, 1024)).astype(
        ml_dtypes.bfloat16)
    qd = np.empty((BH, 128, 2, 1024), np.float32)
    for p in range(2):
        qd[:, 0:32, p, :] = qt[:, 2 * p]
        qd[:, 32:64, p, :] = qt[:, 2 * p + 1]
        qd[:, 64:96, p, :] = A2 * qt[:, 2 * p] ** 2
        qd[:, 96:128, p, :] = A2 * qt[:, 2 * p + 1] ** 2
    qd = np.ascontiguousarray(qd).astype(ml_dtypes.bfloat16)
    kk = kf.reshape(BH, NT, 128, D).transpose(0, 2, 1, 3)  # [bh, p, t, d]
    vvt = vf.reshape(BH, NT, 128, D).transpose(0, 2, 1, 3)
    kpr = np.ascontiguousarray(
        np.concatenate([kk, kk[:, :, :, 0:16], vvt], axis=3)).astype(
            ml_dtypes.bfloat16)
    kgap = np.empty((BH, 128, NT, 5, 32), np.float32)
    for gi_, g_ in enumerate(range(11, 16)):
        kshift = np.concatenate([kk, kk[:, :, :, :16]], axis=3)
        kgap[:, :, :, gi_, :] = kk * kshift[:, :, :, g_:g_ + 32]
    kgap = np.ascontiguousarray(kgap.reshape(BH, 128, NT, 160)).astype(
        ml_dtypes.bfloat16)
    erep, hmtT, isum = _host_consts()
    ones16 = np.zeros((16, 4, 1024), np.float32)
    ones16[0] = 1.0
    ones16 = ones16.astype(ml_dtypes.bfloat16)
    in_maps = []
    for c in range(NCORES):
        sl = slice(c * BH_PER_CORE, (c + 1) * BH_PER_CORE)
        in_maps.append({
            "qt4": np.ascontiguousarray(qt4[sl]),
            "qd": np.ascontiguousarray(qd[sl]),
            "kpr": np.ascontiguousarray(kpr[sl]),
            "kgap": np.ascontiguousarray(kgap[sl]),
            "erep": erep, "hmt": hmtT, "isum": isum,
            "ones1": ones16,
        })
    return in_maps


def _postprocess(res):
    outs = [res.results[c]["out"] for c in range(NCORES)]
    o = np.stack(outs, 0).reshape(B * H, 2, 98, 1024).astype(np.float32)
    ans = np.empty((B * H, 4, E1, 1024), np.float32)
    ans[:, 0::2] = o[:, :, 0:E1, :]
    ans[:, 1::2] = o[:, :, 64:64 + E1, :]
    num = ans[:, :, 0:D, :]
    den = ans[:, :, D:D + 1, :]
    r = (num / den).transpose(0, 1, 3, 2)      # [bh, a, 1024, d]
    return np.ascontiguousarray(r.reshape(B, H, N, D)).astype(np.float32)


def run_traced(q, k, v):
    q = np.ascontiguousarray(np.asarray(q, dtype=np.float32))
    k = np.ascontiguousarray(np.asarray(k, dtype=np.float32))
    v = np.ascontiguousarray(np.asarray(v, dtype=np.float32))
    nc = _get_nc()
    try:
        return run_bass_kernel_spmd(nc, _in_maps(q, k, v),
                                    core_ids=list(range(NCORES)), trace=True)
    except Exception as e:
        print("traced run failed:", e)
        return None


def kernel(q, k, v):
    q = np.ascontiguousarray(np.asarray(q, dtype=np.float32))
    k = np.ascontiguousarray(np.asarray(k, dtype=np.float32))
    v = np.ascontiguousarray(np.asarray(v, dtype=np.float32))
    assert q.shape == (B, H, N, D)
    nc = _get_nc()
    res = run_bass_kernel_spmd(nc, _in_maps(q, k, v),
                               core_ids=list(range(NCORES)))
    return _postprocess(res)


if __name__ == "__main__":
    rng = np.random.default_rng(0)
    q = rng.standard_normal((B, H, N, D), dtype=np.float32)
    k = rng.standard_normal((B, H, N, D), dtype=np.float32)
    v = rng.standard_normal((B, H, N, D), dtype=np.float32)
    o = kernel(q, k, v)
    print("ran", o.shape, o.dtype)


# revision 30
# speedup vs baseline: 1.0675x; 1.0675x over previous
"""FAST multi-head attention (p=2 Taylor linear attention) for Trainium2, v2.

Self-contained: accepts FULL inputs q,k,v [2,16,4096,32] fp32, returns the
full output [2,16,4096,32]. Shards the 32 (b,h) pairs across 8 NeuronCores
(4 per core), one Bass/Tile kernel run SPMD via PJRT.

Per (b,h) (A0=1, A1=1, A2=0.5), with v' = [v | 1]:
  out[n, :] = num / den where [num|den](n, e) = sum_f phi_f(q_n) KV[f, e],
  KV[f, e] = sum_m phi_f(k_m) v'[m, e], over an exact feature basis:
  ones(1) + linear q (32) + diag q^2 (32) + 496 off-diag pair products
  (gaps 1..15 cyclic + half gap 16). Off-diag phi(q) evaluated via the
  square trick phi = (E^T q)^2 with diag contamination removed through
  corrected diag weights wd = KV2dd - Hm @ KVc.

PE array packing (tile_position): kv-side and ans matmuls are (128,64)
col-tiled pairs (2 concurrent M=34 matmuls, one PSUM bank); u = E^T q runs
as (32,64) 8-way tiles (4 a-chunks x 2 feature halves) into a 4-bank PSUM
mega tile. Squares u^2 drain via wide-FD ScalarE ACTIVATE (plus a DVE
copy+mul path) into bf16 phit. Weight prep (p2) uses regular (128,64)
matmuls against a stacked identity, which also merges the two col-tile
partial sums. Interleaved accumulation chains in one bank use start=True
only on the very first matmul of the bank (has_written bits give
overwrite-on-first-touch for the other chain). Host pretransposes q (qt4,
qd incl. A2 q^2 rows) and lays out k with a 16-col wraparound duplicate
for stride-1 DVE products.
"""
import dataclasses
import numpy as np

import concourse.bass as bass
import concourse.tile as tile
from concourse import mybir, bacc
from concourse.bass_utils import run_bass_kernel_spmd

F32 = mybir.dt.float32
BF16 = mybir.dt.bfloat16
SQ = mybir.ActivationFunctionType.Square

A0, A1, A2 = 1.0, 1.0, 0.5
B, H, N, D = 2, 16, 4096, 32
NCORES = 8
BH_PER_CORE = (B * H) // NCORES   # 4
NT = N // 128                     # 32 n-tiles
E1 = 34                           # 32 num + 1 den + 1 pad
NOFF = 496                        # 480 (gaps 1..15) + 16 (half gap 16)

# kt feature-column layout (562 cols):
#  [0:32 k | 32:512 off gaps1-15 | 512:528 half16 | 528:529 ones |
#   529:561 diag | 561:562 pad]
KA_W, KB_W = 288, 274  # ka = kt cols 0:288, kb = kt cols 288:562

# square waves (s,h) routed to the DVE copy+mul path instead of ScalarE
SQ_ON_DVE = set()


def _ap_free(x: bass.AP, free_ap, extra_offset=0):
    return dataclasses.replace(
        x, offset=x.offset + extra_offset,
        ap=[x.ap[0]] + [list(p) for p in free_ap])


def _pairs():
    ps = []
    for g in range(1, 16):
        for d in range(D):
            ps.append((d, (d + g) % D))
    for d in range(16):
        ps.append((d, d + 16))
    return ps


def _host_consts():
    import ml_dtypes
    P = _pairs()
    beta = np.sqrt(A2)
    E = np.zeros((D, NOFF), np.float32)
    Hm = np.zeros((D, NOFF), np.float32)
    for f, (d1, d2) in enumerate(P):
        E[d1, f] += beta
        E[d2, f] += beta
        Hm[d1, f] += 1.0
        Hm[d2, f] += 1.0
    erep = np.zeros((128, 512), np.float32)
    for a in range(4):
        erep[32 * a:32 * a + 32, 0:NOFF] = E
    # hmtT[f', s, 32+d] = Hm[d, 128*s + f']  (d lands at rows 32:64 of out)
    hmtT = np.zeros((128, 4, 64), np.float32)
    for s in range(4):
        w = min(128, NOFF - 128 * s)
        hmtT[0:w, s, 32:64] = -Hm[:, 128 * s:128 * s + w].T
    # isum [128, 64]: rows 0:34 I34, rows 64:98 I34 (merges col-tile halves)
    isum = np.zeros((128, 64), np.float32)
    for m in range(E1):
        isum[m, m] = 1.0
        isum[64 + m, m] = 1.0
    return (erep.astype(ml_dtypes.bfloat16), hmtT.astype(ml_dtypes.bfloat16),
            isum.astype(ml_dtypes.bfloat16))


def build_nc():
    nc = bacc.Bacc(None, target_bir_lowering=False)

    qt4_in = nc.declare_dram_parameter("qt4", [BH_PER_CORE, 128, 1024], BF16,
                                       isOutput=False)
    qd_in = nc.declare_dram_parameter("qd", [BH_PER_CORE, 128, 2, 1024], BF16,
                                      isOutput=False)
    kpr_in = nc.declare_dram_parameter("kpr", [BH_PER_CORE, 128, NT, 80], BF16,
                                       isOutput=False)
    erep_in = nc.declare_dram_parameter("erep", [128, 512], BF16,
                                        isOutput=False)
    hmt_in = nc.declare_dram_parameter("hmt", [128, 4, 64], BF16,
                                       isOutput=False)
    isum_in = nc.declare_dram_parameter("isum", [128, 64], BF16,
                                        isOutput=False)
    ones_in = nc.declare_dram_parameter("ones1", [16, 4, 1024], BF16,
                                        isOutput=False)
    kgap_in = nc.declare_dram_parameter("kgap", [BH_PER_CORE, 128, NT, 160],
                                        BF16, isOutput=False)
    out = nc.declare_dram_parameter("out", [BH_PER_CORE, 2, 98, 1024], BF16,
                                    isOutput=True)

    with tile.TileContext(nc) as tc:
        with (
            tc.tile_pool(name="sb_const", bufs=1) as sb_const,
            tc.tile_pool(name="sb_q", bufs=2) as sb_q,
            tc.tile_pool(name="sb_kp", bufs=2) as sb_kp,
            tc.tile_pool(name="sb_w", bufs=2) as sb_w,
            tc.tile_pool(name="sb_ucp", bufs=2) as sb_ucp,
            tc.tile_pool(name="sb_ev", bufs=2) as sb_ev,
            tc.tile_pool(name="ps_kv", bufs=1, space="PSUM") as ps_kv,
            tc.tile_pool(name="ps_mega", bufs=2, space="PSUM") as ps_mega,
            tc.tile_pool(name="ps_ans", bufs=1, space="PSUM") as ps_ans,
        ):
            # ---- constants ----
            erep = sb_const.tile([128, 512], BF16)
            nc.sync.dma_start(out=erep[:], in_=erep_in[:])
            hmt = sb_const.tile([128, 4, 64], BF16)
            nc.sync.dma_start(out=hmt[:], in_=hmt_in[:])
            isum = sb_const.tile([128, 64], BF16)
            nc.sync.dma_start(out=isum[:], in_=isum_in[:])

            kts = [sb_const.tile([128, 8, 562], BF16, name=f"kts{i}")
                   for i in range(4)]
            vxs = [sb_const.tile([128, 8, E1], BF16, name=f"vxs{i}")
                   for i in range(4)]
            for i in range(4):
                nc.gpsimd.memset(kts[i][:, :, 528:529], 1.0)
                nc.gpsimd.memset(kts[i][:, :, 561:562], 0.0)
                nc.gpsimd.memset(vxs[i][:, :, 32:34], 1.0)
            phits = [sb_const.tile([128, 4, 4, 1024], BF16, name=f"phit{i}")
                     for i in range(2)]
            for i in range(2):
                nc.sync.dma_start(out=phits[i][112:128, :, 3, :],
                                  in_=ones_in[:])
            wqd_ev = sb_const.tile([128, E1], BF16, name="wqd_ev")
            wqd_od = sb_const.tile([128, E1], BF16, name="wqd_od")
            nc.gpsimd.memset(wqd_ev[:], 0.0)
            nc.gpsimd.memset(wqd_od[:], 0.0)

            def ktvx(b, g):
                return kts[g], vxs[g]

            def emit_kgap(b, g_):
                kt, _ = ktvx(b, g_)
                nc.sync.dma_start(out=kt[:, :, 352:512],
                                  in_=kgap_in[b][:, 8 * g_:8 * g_ + 8, :])

            def emit_dma_in(b, first=False):
                st = dict(b=b, phit=phits[b % 2])
                st["kpr"] = sb_kp.tile([128, NT, 80], BF16, tag="kpr",
                                       name=f"kpr_{b}")
                for g_ in range(4):
                    ts8 = slice(8 * g_, 8 * g_ + 8)
                    nc.sync.dma_start(out=st["kpr"][:, ts8, :],
                                      in_=kpr_in[b][:, ts8, :])
                    if first:
                        emit_kgap(b, g_)
                st["qt4"] = sb_q.tile([128, 1024], BF16, tag="qt4",
                                      name=f"qt4_{b}")
                nc.gpsimd.dma_start(out=st["qt4"][:], in_=qt4_in[b])
                st["qd"] = sb_q.tile([128, 2, 1024], BF16, tag="qd",
                                     name=f"qd_{b}")
                nc.gpsimd.dma_start(out=st["qd"][:], in_=qd_in[b])
                st["ka"] = ps_kv.tile([128, KA_W], F32, tag="ka",
                                      name=f"ka{b}")
                st["kb"] = ps_kv.tile([128, KB_W], F32, tag="kb",
                                      name=f"kb{b}")
                return st

            def emit_p1_group(cur, g):
                b = cur["b"]
                kt, vx = ktvx(b, g)
                ts8 = slice(8 * g, 8 * g + 8)
                kp = cur["kpr"]
                nc.vector.tensor_copy(kt[:, :, 0:32], kp[:, ts8, 0:32])
                nc.vector.tensor_copy(vx[:, :, 0:32], kp[:, ts8, 48:80])
                # off-diag products, gaps 1..15 -> kt cols 32:512
                nc.vector.tensor_mul(
                    _ap_free(kt[:, 0, 32:352], [[562, 8], [32, 10], [1, 32]]),
                    _ap_free(kp[:, 8 * g, 0:32], [[80, 8], [0, 10], [1, 32]]),
                    _ap_free(kp[:, 8 * g, 1:33], [[80, 8], [1, 10], [1, 32]]))
                # half gap 16 -> cols 512:528 ; diag -> cols 529:561
                nc.vector.tensor_mul(kt[:, :, 512:528], kp[:, ts8, 0:16],
                                     kp[:, ts8, 16:32])
                nc.vector.tensor_mul(kt[:, :, 529:561], kp[:, ts8, 0:32],
                                     kp[:, ts8, 0:32])
                for tt in range(8):
                    t = 8 * g + tt
                    par = (t % 2) * 64
                    st_, sp = (t < 2), (t >= NT - 2)
                    nc.tensor.matmul(cur["ka"][par:par + E1, :],
                                     vx[:, tt, :], kt[:, tt, 0:KA_W],
                                     start=st_, stop=sp,
                                     skip_group_check=True)
                    nc.tensor.matmul(cur["kb"][par:par + E1, :],
                                     vx[:, tt, :], kt[:, tt, KA_W:562],
                                     start=st_, stop=sp,
                                     skip_group_check=True)

            def emit_u_group(cur, s, h):
                b = cur["b"]
                qt4 = cur["qt4"]
                hs = slice(512 * h, 512 * h + 512)
                pp = 128 if s < 3 else 112
                phit = cur["phit"]
                for ap_ in range(2):
                    mega = ps_mega.tile([128, 1024], F32, tag="mega",
                                        name=f"mega{b}_{s}_{h}_{ap_}")
                    for ci in range(2):
                        a = 2 * ap_ + ci
                        for fh in range(2):
                            fw = 64 if (s < 3 or fh == 0) else 48
                            fc = 128 * s + 64 * fh
                            nc.tensor.matmul(
                                mega[64 * fh:64 * fh + fw,
                                     512 * ci:512 * ci + 512],
                                erep[32 * a:32 * a + 32, fc:fc + fw],
                                qt4[32 * a:32 * a + 32, hs],
                                start=True, stop=True,
                                tile_position=(32 * a, 64 * fh),
                                skip_group_check=True)
                    dst = _ap_free(
                        phit[0:pp, 2 * ap_, s, 512 * h:512 * h + 512],
                        [[4096, 2], [1, 512]])
                    src = _ap_free(mega[0:pp, 0:512], [[512, 2], [1, 512]])
                    if (s, h, ap_) in SQ_ON_DVE:
                        ucp = sb_ucp.tile([128, 1024], BF16, tag="ucp",
                                          name=f"ucp{b}_{s}_{h}_{ap_}")
                        nc.vector.tensor_copy(ucp[0:pp, :], mega[0:pp, :])
                        us = _ap_free(ucp[0:pp, 0:512], [[512, 2], [1, 512]])
                        nc.vector.tensor_mul(dst, us, us)
                    else:
                        nc.scalar.activation(out=dst, in_=src, func=SQ,
                                             scale=1.0)

            def emit_ans_chunk(st, pair, h):
                b = st["b"]
                if h == 0:
                    st["ansm%d" % pair] = ps_ans.tile(
                        [128, 1024], F32, tag="ansm", name=f"ansm{b}_{pair}")
                ansm = st["ansm%d" % pair]
                hs = slice(512 * h, 512 * h + 512)
                phit = st["phit"]
                wc = st["wc"]
                for s in range(4):
                    for ci in range(2):
                        a = 2 * pair + ci
                        nc.tensor.matmul(
                            ansm[64 * ci:64 * ci + E1, hs],
                            wc[:, 34 * s:34 * s + 34], phit[:, a, s, hs],
                            start=(s == 0), stop=False,
                            skip_group_check=True)
                for ci in range(2):
                    nc.tensor.matmul(
                        ansm[64 * ci:64 * ci + E1, hs],
                        wqd_ev if ci == 0 else wqd_od,
                        st["qd"][:, pair, hs],
                        start=False, stop=(ci == 1), skip_group_check=True)

            def emit_evict(st, pair):
                b = st["b"]
                ev = sb_ev.tile([128, 1024], BF16, tag="ev",
                                name=f"ev{b}_{pair}")
                nc.vector.tensor_copy(ev[0:98, :], st["ansm%d" % pair][0:98, :])
                nc.gpsimd.dma_start(out=out[b][pair], in_=ev[0:98, :])

            def emit_p2(cur):
                b = cur["b"]
                cov = sb_w.tile([128, KA_W], BF16, tag="cov", name=f"cov{b}")
                nc.vector.tensor_copy(cov[:], cur["ka"][:])
                cov2 = sb_w.tile([128, KB_W], BF16, tag="cov2",
                                 name=f"cov2{b}")
                nc.vector.tensor_copy(cov2[:], cur["kb"][:])

                wcp = ps_mega.tile([128, 1024], F32, tag="mega",
                                  name=f"wcp{b}")
                # wc chunks (merged col-tile halves via isum):
                srcs = [(cov, 32), (cov, 96), (cov, 160), (cov, 224),
                        (cov2, 0), (cov2, 64), (cov2, 128), (cov2, 192)]
                for i, (sc, c0) in enumerate(srcs):
                    s_, fh = i // 2, i % 2
                    w = 49 if i == 7 else 64
                    nc.tensor.matmul(wcp[64 * fh:64 * fh + w,
                                         34 * s_:34 * s_ + 34],
                                     sc[:, c0:c0 + w], isum[:, 0:34],
                                     start=True, stop=False,
                                     skip_group_check=True)
                # wq rows 0:32 at cols 170:204 ; diagT rows 32:64 at 204:238
                nc.tensor.matmul(wcp[0:64, 170:204], cov[:, 0:64],
                                 isum[:, 0:34], start=True, stop=False,
                                 skip_group_check=True)
                nc.tensor.matmul(wcp[0:64, 204:238], cov2[:, 209:273],
                                 isum[:, 0:34], start=True, stop=False,
                                 skip_group_check=True)
                wc = sb_w.tile([128, 136], BF16, tag="wc", name=f"wc{b}")
                nc.vector.tensor_copy(wc[:], wcp[:, 0:136])
                # wd = diagT - Hm @ KVc: hmt is negated, accumulate onto diag
                for s_ in range(4):
                    nc.tensor.matmul(wcp[0:64, 204:238], hmt[:, s_, :],
                                     wc[:, 34 * s_:34 * s_ + 34],
                                     start=False, stop=(s_ == 3),
                                     skip_group_check=True)
                wdt = sb_w.tile([64, E1], BF16, tag="wdt", name=f"wdt{b}")
                nc.vector.tensor_copy(wdt[32:64, :], wcp[32:64, 204:238])
                wqs = sb_w.tile([32, E1], BF16, tag="wqs", name=f"wqs{b}")
                nc.vector.tensor_copy(wqs[:], wcp[0:32, 170:204])
                cur["wdt"] = wdt
                cur["wqs"] = wqs
                cur["wcp"] = wcp
                cur["wc"] = wc

            def emit_p2b(cur):
                # assemble wqd masks (after prev's last ans chunk read them)
                nc.vector.tensor_copy(wqd_ev[0:32, :], cur["wcp"][0:32,
                                                                  170:204])
                nc.gpsimd.dma_start(out=wqd_od[32:64, :], in_=cur["wqs"][:])
                nc.gpsimd.dma_start(out=wqd_ev[64:96, :],
                                    in_=cur["wdt"][32:64, :])
                nc.gpsimd.dma_start(out=wqd_od[96:128, :],
                                    in_=cur["wdt"][32:64, :])

            # ---------------- main pipeline over b ----------------
            prev = None
            nxt = emit_dma_in(0, first=True)
            for b in range(BH_PER_CORE + 1):
                cur = nxt if b < BH_PER_CORE else None
                nxt = None
                for gi in range(8):
                    g, h = gi // 2, gi % 2
                    if cur is not None and h == 0:
                        emit_p1_group(cur, g)
                    if prev is not None and h == 1:
                        c = gi // 2
                        emit_ans_chunk(prev, c // 2, c % 2)
                        if c % 2 == 1:
                            emit_evict(prev, c // 2)
                    if cur is not None:
                        emit_u_group(cur, g, h)
                    if cur is not None and gi == 6:
                        emit_p2(cur)
                    if cur is not None and b + 1 < BH_PER_CORE:
                        if gi == 3:
                            nxt = emit_dma_in(b + 1)
                            emit_kgap(b + 1, 0)
                            emit_kgap(b + 1, 1)
                        elif gi == 5:
                            emit_kgap(b + 1, 2)
                        elif gi == 7:
                            emit_kgap(b + 1, 3)
                if cur is not None:
                    emit_p2b(cur)
                prev = cur

    nc.compile()
    return nc


_NC_CACHE = None


def _get_nc():
    global _NC_CACHE
    if _NC_CACHE is None:
        _NC_CACHE = build_nc()
    return _NC_CACHE


def _in_maps(q, k, v):
    import ml_dtypes
    BH = B * H
    qf = q.reshape(BH, N, D)
    kf = k.reshape(BH, N, D)
    vf = v.reshape(BH, N, D)
    qt = qf.reshape(BH, 4, 1024, D).transpose(0, 1, 3, 2)  # [bh, a, d, 1024]
    qt4 = np.ascontiguousarray(qt.reshape(BH, 128, 1024)).astype(
        ml_dtypes.bfloat16)
    qd = np.empty((BH, 128, 2, 1024), np.float32)
    for p in range(2):
        qd[:, 0:32, p, :] = qt[:, 2 * p]
        qd[:, 32:64, p, :] = qt[:, 2 * p + 1]
        qd[:, 64:96, p, :] = A2 * qt[:, 2 * p] ** 2
        qd[:, 96:128, p, :] = A2 * qt[:, 2 * p + 1] ** 2
    qd = np.ascontiguousarray(qd).astype(ml_dtypes.bfloat16)
    kk = kf.reshape(BH, NT, 128, D).transpose(0, 2, 1, 3)  # [bh, p, t, d]
    vvt = vf.reshape(BH, NT, 128, D).transpose(0, 2, 1, 3)
    kpr = np.ascontiguousarray(
        np.concatenate([kk, kk[:, :, :, 0:16], vvt], axis=3)).astype(
            ml_dtypes.bfloat16)
    kgap = np.empty((BH, 128, NT, 5, 32), np.float32)
    for gi_, g_ in enumerate(range(11, 16)):
        kshift = np.concatenate([kk, kk[:, :, :, :16]], axis=3)
        kgap[:, :, :, gi_, :] = kk * kshift[:, :, :, g_:g_ + 32]
    kgap = np.ascontiguousarray(kgap.reshape(BH, 128, NT, 160)).astype(
        ml_dtypes.bfloat16)
    erep, hmtT, isum = _host_consts()
    ones16 = np.zeros((16, 4, 1024), np.float32)
    ones16[0] = 1.0
    ones16 = ones16.astype(ml_dtypes.bfloat16)
    in_maps = []
    for c in range(NCORES):
        sl = slice(c * BH_PER_CORE, (c + 1) * BH_PER_CORE)
        in_maps.append({
            "qt4": np.ascontiguousarray(qt4[sl]),
            "qd": np.ascontiguousarray(qd[sl]),
            "kpr": np.ascontiguousarray(kpr[sl]),
            "kgap": np.ascontiguousarray(kgap[sl]),
            "erep": erep, "hmt": hmtT, "isum": isum,
            "ones1": ones16,
        })
    return in_maps


def _postprocess(res):
    outs = [res.results[c]["out"] for c in range(NCORES)]
    o = np.stack(outs, 0).reshape(B * H, 2, 98, 1024).astype(np.float32)
    ans = np.empty((B * H, 4, E1, 1024), np.float32)
    ans[:, 0::2] = o[:, :, 0:E1, :]
    ans[:, 1::2] = o[:, :, 64:64 + E1, :]
    num = ans[:, :, 0:D, :]
    den = ans[:, :, D:D + 1, :]
    r = (num / den).transpose(0, 1, 3, 2)      # [bh, a, 1024, d]
    return np.ascontiguousarray(r.reshape(B, H, N, D)).astype(np.float32)


def run_traced(q, k, v):
    q = np.ascontiguousarray(np.asarray(q, dtype=np.float32))
    k = np.ascontiguousarray(np.asarray(k, dtype=np.float32))
    v = np.ascontiguousarray(np.asarray(v, dtype=np.float32))
    nc = _get_nc()
    try:
        return run_bass_kernel_spmd(nc, _in_maps(q, k, v),
                                    core_ids=list(range(NCORES)), trace=True)
    except Exception as e:
        print("traced run failed:", e)
        return None


def kernel(q, k, v):
    q = np.ascontiguousarray(np.asarray(q, dtype=np.float32))
    k = np.ascontiguousarray(np.asarray(k, dtype=np.float32))
    v = np.ascontiguousarray(np.asarray(v, dtype=np.float32))
    assert q.shape == (B, H, N, D)
    nc = _get_nc()
    res = run_bass_kernel_spmd(nc, _in_maps(q, k, v),
                               core_ids=list(range(NCORES)))
    return _postprocess(res)


if __name__ == "__main__":
    rng = np.random.default_rng(0)
    q = rng.standard_normal((B, H, N, D), dtype=np.float32)
    k = rng.standard_normal((B, H, N, D), dtype=np.float32)
    v = rng.standard_normal((B, H, N, D), dtype=np.float32)
    o = kernel(q, k, v)
    print("ran", o.shape, o.dtype)
